# revision 52
# baseline (speedup 1.0000x reference)
"""Self-contained Trainium2 Bass kernel for EnhancedAutoformerAttention.

Sharding: core c handles batch b=c//2, query half qh=c%2 (1024 queries, all
8 heads). No cross-core reduction; host gather is a pure reshape.

v2 design (ScalarE-exp-bound):
  - Host pre-transposes and pre-casts everything the PE would otherwise
    transpose on chip: W*^T, Q^T/K^T/V^T and the mask arrive as bf16 in the
    exact SBUF layouts the matmuls consume.  No on-chip transposes.
  - ScalarE does ONLY sigmoid (prologue) + the attention exp stream; all
    projection epilogues run on VectorE (tensor_scalar), so the exp stream
    is never interrupted and the ACT table is loaded once.
  - Attention is software-pipelined per (head-pair, kc):
      issue order QK(h0) | PV(prev,h0) | QK(h1) | PV(prev,h1) | exp x2 |
      mask-mult x2 keeps exp back-to-back with 2 score PSUM buffers.
  - Softmax denominators: ones-column in the PV lhsT; reciprocal via the
    DVE approx op (no ScalarE Ln/Exp, no table switches); ctx divide and
    evacuation work is deferred into the next head-pair's loop slack.
"""

from contextlib import ExitStack

import numpy as np
import ml_dtypes

import concourse.bass as bass
import concourse.mybir as mybir
import concourse.tile as tile
from concourse import bacc
from concourse.bass_utils import run_bass_kernel_spmd

dt = mybir.dt
F32, BF16, I32 = dt.float32, dt.bfloat16, dt.int32
AF = mybir.ActivationFunctionType
OP = mybir.AluOpType
BF = ml_dtypes.bfloat16

B, S, D, H, DK = 4, 2048, 512, 8, 64
LN_EPS = 1e-5
N_CORES = 8
S_q, S_kv = 1024, 2048
DC = D // 128          # 4 feature-dim chunks
KC = S_kv // 128       # 16 k-position tiles
ST = S_kv // 128       # 16 v tiles
QT = S_q // 128        # 8 out q tiles
HP = H // 2            # 4 head pairs


def bcast_ap(src: bass.AP, p: int) -> bass.AP:
    """AP reading src (partition dim 1) broadcast to p partitions."""
    return bass.AP(tensor=src.tensor, offset=src.offset,
                   ap=[[0, p]] + list(src.ap[1:]))


def build_kernel(n_devices: int = N_CORES,
                 ln_identity: bool = False, zero_bo: bool = False,
                 dbg: bool = False):
    nc = bacc.Bacc("TRN2", target_bir_lowering=False, debug=False,
                   num_devices=n_devices)
    ein = dict(kind="ExternalInput")
    QTs = nc.dram_tensor("QTs", [D, S_q], BF16, **ein).ap()
    KTs = nc.dram_tensor("KTs", [D, S_kv], BF16, **ein).ap()
    VTs = nc.dram_tensor("VTs", [D, S_kv], BF16, **ein).ap()
    MTs = nc.dram_tensor("MTs", [S_kv, S_q], BF16, **ein).ap()
    Wd = {w: nc.dram_tensor(w, [D, D], BF16, **ein).ap()
          for w in ("WqT", "WkT", "WvT", "WoT")}
    qaddv = nc.dram_tensor("qaddv", [D], F32, **ein).ap()    # bq + time_w
    bkiv = nc.dram_tensor("bkiv", [D], F32, **ein).ap()      # bk / temp
    invtv = nc.dram_tensor("invtv", [128], F32, **ein).ap()  # 1/temp rep
    gw2 = nc.dram_tensor("gw2", [128], F32, **ein).ap()      # gate_w x2
    gb = nc.dram_tensor("gb", [1], F32, **ein).ap()
    boeff = nc.dram_tensor("boeff", [D], F32, **ein).ap()    # bo + Wo@bv
    lngv = nc.dram_tensor("lngv", [D], F32, **ein).ap()
    lnbv = nc.dram_tensor("lnbv", [D], F32, **ein).ap()
    out = nc.dram_tensor("out", [S_q, D], F32, kind="ExternalOutput").ap()
    dbgt = None
    if dbg:
        eo = dict(kind="ExternalOutput")
        dbgt = {
            "d_qTg": nc.dram_tensor("d_qTg", [128, DC, S_q], BF16, **eo).ap(),
            "d_kT": nc.dram_tensor("d_kT", [128, DC, S_kv], BF16, **eo).ap(),
            "d_vsb": nc.dram_tensor("d_vsb", [128, ST, H, 65], BF16,
                                    **eo).ap(),
            "d_p": nc.dram_tensor("d_p", [128, 2, S_q], BF16, **eo).ap(),
            "d_pm": nc.dram_tensor("d_pm", [128, 2, S_q], BF16, **eo).ap(),
            "d_cf": nc.dram_tensor("d_cf", [2, 65, S_q], F32, **eo).ap(),
            "d_rb": nc.dram_tensor("d_rb", [128, S_q], F32, **eo).ap(),
            "d_ctxT": nc.dram_tensor("d_ctxT", [128, DC, S_q], BF16,
                                     **eo).ap(),
        }

    with tile.TileContext(nc) as tc:
        _body(nc, tc, QTs, KTs, VTs, MTs, Wd, qaddv, bkiv, invtv, gw2, gb,
              boeff, lngv, lnbv, out, ln_identity, zero_bo, dbgt)
    nc.compile()
    return nc


def _body(nc, tc, QTs, KTs, VTs, MTs, Wd, qaddv, bkiv, invtv, gw2, gb,
          boeff, lngv, lnbv, out, ln_identity, zero_bo, dbgt=None):
    with (
        tc.tile_pool(name="persist", bufs=1) as per,
        tc.tile_pool(name="cols", bufs=1) as cols,
    ):
        # ---- small constants (scalar HWDGE queue; sync carries Q/K) ----
        qadd = cols.tile([128, DC], F32, tag="qadd")
        nc.scalar.dma_start(out=qadd,
                            in_=qaddv.rearrange("(c p) -> p c", p=128))
        bkic = cols.tile([128, DC], F32, tag="bkic")
        nc.scalar.dma_start(out=bkic,
                            in_=bkiv.rearrange("(c p) -> p c", p=128))
        invt = cols.tile([128, 1], F32, tag="invt")
        nc.scalar.dma_start(out=invt,
                            in_=invtv.rearrange("(p o) -> p o", o=1))
        gbc = cols.tile([128, 1], F32, tag="gbc")
        nc.scalar.dma_start(out=gbc, in_=bcast_ap(gb[None, :], 128))
        epsc = cols.tile([128, 1], F32, tag="epsc")
        nc.vector.memset(epsc, LN_EPS)

        # block-diag gate weights (broadcasts gate logit to 64 rows/head)
        gwcol = cols.tile([128, 1], F32, tag="gwcol")
        nc.scalar.dma_start(out=gwcol, in_=gw2.rearrange("(p o) -> p o", o=1))
        ones_bd = cols.tile([128, 128], BF16, tag="ones_bd")
        nc.vector.memset(ones_bd, 0.0)
        nc.vector.memset(ones_bd[0:64, 0:64], 1.0)
        nc.vector.memset(ones_bd[64:128, 64:128], 1.0)
        gwrep = cols.tile([128, 128], BF16, tag="gwrep")
        nc.vector.tensor_scalar_mul(gwrep, ones_bd, gwcol)

        if not ln_identity:
            lng_b = per.tile([128, D], F32, tag="lngb")
            nc.scalar.dma_start(out=lng_b, in_=bcast_ap(lngv[None, :], 128))
            lnb_b = per.tile([128, D], F32, tag="lnbb")
            nc.scalar.dma_start(out=lnb_b, in_=bcast_ap(lnbv[None, :], 128))
        if not zero_bo:
            onesrow = cols.tile([128, 128], BF16, tag="onesrow")
            nc.vector.memset(onesrow, 0.0)
            nc.vector.memset(onesrow[0:1, :], 1.0)
            bo_pad = cols.tile([128, D], BF16, tag="bo_pad")
            nc.vector.memset(bo_pad, 0.0)
            borow = cols.tile([1, D], F32, tag="borow")
            nc.scalar.dma_start(out=borow, in_=boeff[None, :])
            nc.vector.tensor_copy(out=bo_pad[0:1, :], in_=borow)

        # ---- persistent big tensors ------------------------------------
        WT = {w: per.tile([128, DC, D], BF16, tag=w, name=w) for w in Wd}
        maskT = per.tile([128, KC, S_q], BF16, tag="maskT")
        kT = per.tile([128, DC, S_kv], BF16, tag="kT")
        v_sb = per.tile([128, ST, H, 65], BF16, tag="v_sb")
        qT = per.tile([128, DC, S_q], BF16, tag="qT")
        qTg = per.tile([128, DC, S_q], BF16, tag="qTg")
        ctxTr = per.tile([128, DC, S_q], BF16, tag="ctxTr")
        ctxT = per.tile([128, DC, S_q], BF16, tag="ctxT")

        nc.gpsimd.memset(v_sb[:, :, :, 64:65], 1.0)

        # ---- DMAs: everything, biggest-priority-first ------------------
        nc.sync.dma_start(out=WT["WqT"],
                          in_=Wd["WqT"].rearrange("(c p) d -> p c d", p=128))
        att = ExitStack()   # holds psS across the stage-pool close
        with tc.tile_pool(name="stage", bufs=1) as stg:
            QT_s = stg.tile([128, DC, S_q], BF16, tag="QT_s")
            KT_s = stg.tile([128, DC, S_kv], BF16, tag="KT_s")
            VT_s = stg.tile([128, DC, S_kv], BF16, tag="VT_s")
            for sc in range(2):
                sl = slice(sc * 512, (sc + 1) * 512)
                nc.sync.dma_start(
                    out=QT_s[:, :, sl],
                    in_=QTs[:, sl].rearrange("(c p) q -> p c q", p=128))
            nc.scalar.dma_start(
                out=WT["WkT"],
                in_=Wd["WkT"].rearrange("(c p) d -> p c d", p=128))
            for sc in range(4):
                sl = slice(sc * 512, (sc + 1) * 512)
                nc.sync.dma_start(
                    out=KT_s[:, :, sl],
                    in_=KTs[:, sl].rearrange("(c p) s -> p c s", p=128))
            nc.scalar.dma_start(
                out=WT["WvT"],
                in_=Wd["WvT"].rearrange("(c p) d -> p c d", p=128))
            for sc in range(4):
                sl = slice(sc * 512, (sc + 1) * 512)
                nc.sync.dma_start(
                    out=VT_s[:, :, sl],
                    in_=VTs[:, sl].rearrange("(c p) s -> p c s", p=128))
            for g in range(4):
                sl = slice(g * 4, (g + 1) * 4)
                nc.sync.dma_start(
                    out=maskT[:, sl, :],
                    in_=MTs[g * 512:(g + 1) * 512, :].rearrange(
                        "(kc p) q -> p kc q", p=128))
            nc.scalar.dma_start(
                out=WT["WoT"],
                in_=Wd["WoT"].rearrange("(c p) d -> p c d", p=128))

            # ---- projections (prologue; epilogues on VectorE) ----------
            with (
                tc.tile_pool(name="psP", bufs=4, space="PSUM") as psP,
                tc.tile_pool(name="gpool", bufs=2) as gp,
            ):
                # Q projection -> qT (+bq+time_w)
                for sc in range(2):
                    sl = slice(sc * 512, (sc + 1) * 512)
                    for c in range(DC):
                        pq = psP.tile([128, 512], F32, tag="pp", name="pq")
                        for Dc in range(DC):
                            nc.tensor.matmul(
                                pq, lhsT=WT["WqT"][:, Dc, c * 128:(c + 1) * 128],
                                rhs=QT_s[:, Dc, sl],
                                start=(Dc == 0), stop=(Dc == DC - 1))
                        nc.vector.tensor_scalar_add(
                            out=qT[:, c, sl], in0=pq, scalar1=qadd[:, c:c + 1])
                # gate: sigmoid(gw.q + gb) broadcast over dk rows, fold into q
                for c in range(DC):
                    pg = psP.tile([128, S_q], F32, tag="pg", name="pg", bufs=1)
                    for j in range(2):
                        js = slice(j * 512, (j + 1) * 512)
                        nc.tensor.matmul(pg[:, js], lhsT=gwrep,
                                         rhs=qT[:, c, js], start=True, stop=True)
                    gbf = gp.tile([128, S_q], BF16, tag="gbf", name="gbf")
                    nc.scalar.activation(out=gbf, in_=pg, func=AF.Sigmoid,
                                         bias=gbc, scale=1.0)
                    nc.vector.tensor_mul(qTg[:, c, :], qT[:, c, :], gbf)
                # dummy exp: hoists the EXP ACT-table load into the prologue
                scr1 = cols.tile([128, 1], F32, tag="scr1")
                nc.scalar.activation(out=scr1, in_=epsc, func=AF.Exp)
                # K projection -> kT ((k+bk)/temp)
                for sc in range(4):
                    sl = slice(sc * 512, (sc + 1) * 512)
                    for c in range(DC):
                        pk = psP.tile([128, 512], F32, tag="pp", name="pk")
                        for Dc in range(DC):
                            nc.tensor.matmul(
                                pk, lhsT=WT["WkT"][:, Dc, c * 128:(c + 1) * 128],
                                rhs=KT_s[:, Dc, sl],
                                start=(Dc == 0), stop=(Dc == DC - 1))
                        nc.vector.tensor_scalar(
                            out=kT[:, c, sl], in0=pk,
                            scalar1=invt, scalar2=bkic[:, c:c + 1],
                            op0=OP.mult, op1=OP.add)
                # V projection -> v_sb natural [s, h, dk] (bv folded into bo)
                for st in range(ST - 4):
                    pv = psP.tile([128, 512], F32, tag="pp", name="pv")
                    for Dc in range(DC):
                        nc.tensor.matmul(
                            pv, lhsT=VT_s[:, Dc, st * 128:(st + 1) * 128],
                            rhs=WT["WvT"][:, Dc, :],
                            start=(Dc == 0), stop=(Dc == DC - 1))
                    nc.vector.tensor_copy(
                        out=v_sb[:, st, :, 0:64],
                        in_=pv.rearrange("p (h d) -> p h d", h=H))

            # prologue PSUM is now free: open the score pool early and run
            # the last 4 V chunks in its banks — a ~3.4us dense matmul burst
            # (the PE warmup) that is real work instead of dummies
            psS = att.enter_context(
                tc.tile_pool(name="psS", bufs=1, space="PSUM"))
            vsc = [psS.tile([128, S_q], F32, tag=f"sc{i}", name=f"vsc{i}")
                   for i in range(2)]
            for i, st in enumerate(range(ST - 4, ST)):
                sl = vsc[i % 2][:, (i // 2) * 512:(i // 2 + 1) * 512]
                for Dc in range(DC):
                    nc.tensor.matmul(
                        sl, lhsT=VT_s[:, Dc, st * 128:(st + 1) * 128],
                        rhs=WT["WvT"][:, Dc, :],
                        start=(Dc == 0), stop=(Dc == DC - 1))
                nc.vector.tensor_copy(
                    out=v_sb[:, st, :, 0:64],
                    in_=sl.rearrange("p (h d) -> p h d", h=H))

        if dbgt is not None:
            nc.sync.dma_start(out=dbgt["d_qTg"], in_=qTg)
            nc.sync.dma_start(out=dbgt["d_kT"], in_=kT)
            nc.sync.dma_start(out=dbgt["d_vsb"], in_=v_sb)

        # ---- attention core (exp-paced pipeline) -----------------------
        with (
            tc.tile_pool(name="psC", bufs=1, space="PSUM") as psC,
            tc.tile_pool(name="pp", bufs=4) as pp,
            tc.tile_pool(name="rp", bufs=2) as rp,
            tc.tile_pool(name="rd", bufs=2, space="DRAM") as rd,
        ):
            pending = []   # deferred DVE/DMA work, drained 1 task per kc

            def drain(n):
                for _ in range(min(n, len(pending))):
                    pending.pop(0)()

            for hp in range(HP):
                ctx2 = [psC.tile([65, S_q], F32, tag=f"ctx{i}", name=f"ctx{i}")
                        for i in range(2)]
                pm_prev = None
                for kc in range(KC):
                    sc2 = [psS.tile([128, S_q], F32, tag=f"sc{i}",
                                    name=f"sc{i}") for i in range(2)]
                    kcs = slice(kc * 128, (kc + 1) * 128)
                    # PE: QK(h0) | PV(prev,h0) | QK(h1) | PV(prev,h1)
                    for half in range(2):
                        hs = slice(half * 64, (half + 1) * 64)
                        for j in range(2):
                            js = slice(j * 512, (j + 1) * 512)
                            nc.tensor.matmul(
                                sc2[half][:, js], lhsT=kT[hs, hp, kcs],
                                rhs=qTg[hs, hp, js], start=True, stop=True)
                        if pm_prev is not None:
                            for j in range(2):
                                js = slice(j * 512, (j + 1) * 512)
                                nc.tensor.matmul(
                                    ctx2[half][:, js],
                                    lhsT=v_sb[:, kc - 1, 2 * hp + half, :],
                                    rhs=pm_prev[half][:, js],
                                    start=(kc - 1 == 0), stop=(kc - 1 == KC - 1))
                    # ScalarE: exp
                    p2 = [pp.tile([128, S_q], BF16, tag=f"p{i}", name=f"p{i}",
                                  bufs=6) for i in range(2)]
                    pm2 = [pp.tile([128, S_q], BF16, tag=f"pm{i}",
                                   name=f"pm{i}", bufs=6) for i in range(2)]
                    for half in range(2):
                        nc.scalar.activation(out=p2[half], in_=sc2[half],
                                             func=AF.Exp)
                        nc.vector.tensor_mul(pm2[half], p2[half],
                                             maskT[:, kc, :])
                    if dbgt is not None and hp == 0 and kc == 0:
                        for half in range(2):
                            nc.sync.dma_start(out=dbgt["d_p"][:, half, :],
                                              in_=p2[half])
                            nc.sync.dma_start(out=dbgt["d_pm"][:, half, :],
                                              in_=pm2[half])
                    pm_prev = pm2
                    drain(1)
                # final PV for kc=KC-1
                for half in range(2):
                    for j in range(2):
                        js = slice(j * 512, (j + 1) * 512)
                        nc.tensor.matmul(
                            ctx2[half][:, js],
                            lhsT=v_sb[:, KC - 1, 2 * hp + half, :],
                            rhs=pm_prev[half][:, js],
                            start=False, stop=True)

                # evacuate ctx+den, reciprocal, divide — deferred into the
                # next head-pair's loop slack (DVE/DMA only, no ScalarE)
                def make_evac(hp, ctx2):
                    # ctx PSUM must be read out NOW: the next head-pair's
                    # first PV (start=True) reuses these banks, and a
                    # deferred reader issued after that writer would race.
                    cf = []
                    rdt = rd.tile([2, S_q], F32, tag="rdt", name="rdt")
                    for half in range(2):
                        c = rp.tile([65, S_q], F32, tag=f"cf{half}", name="cf")
                        if hp == HP - 1 and half == 0:
                            # ScalarE is free after the last exp — parallel
                            # evacuation shortens the tail chain
                            nc.scalar.copy(out=c, in_=ctx2[half][0:65, :])
                        else:
                            nc.vector.tensor_copy(out=c,
                                                  in_=ctx2[half][0:65, :])
                        cf.append(c)
                        # den row to DRAM as soon as this half is evacuated
                        nc.sync.dma_start(out=rdt[half:half + 1, :],
                                          in_=c[64:65, :])
                    if dbgt is not None and hp == 0:
                        for half in range(2):
                            nc.sync.dma_start(out=dbgt["d_cf"][half],
                                              in_=cf[half])
                    ctmp = [None]

                    def ev_ctx0():
                        nc.vector.tensor_copy(out=ctxTr[0:64, hp, :],
                                              in_=cf[0][0:64, :])

                    def ev_ctx1():
                        ctmp[0] = rp.tile([64, S_q], BF16, tag="ctmp",
                                          name="ctmp")
                        nc.vector.tensor_copy(out=ctmp[0], in_=cf[1][0:64, :])

                    def ev_ctx1b():
                        nc.sync.dma_start(out=ctxTr[64:128, hp, :], in_=ctmp[0])

                    def ev_rb():
                        rb = rp.tile([128, S_q], F32, tag="rb", name="rb",
                                     bufs=1)
                        nc.sync.dma_start(out=rb[0:64, :],
                                          in_=bcast_ap(rdt[0:1, :], 64))
                        nc.sync.dma_start(out=rb[64:128, :],
                                          in_=bcast_ap(rdt[1:2, :], 64))
                        cf.append(rb)

                    def ev_div():
                        rb = cf[2]
                        rbr = rp.tile([128, S_q], F32, tag="rbr", name="rbr",
                                      bufs=1)
                        nc.vector.reciprocal_approx_fast(out=rbr, in_=rb)
                        nc.vector.tensor_mul(ctxT[:, hp, :], ctxTr[:, hp, :],
                                             rbr)
                        if dbgt is not None and hp == 0:
                            nc.sync.dma_start(out=dbgt["d_rb"], in_=rbr)
                    return [ev_rb, ev_ctx0, ev_ctx1, ev_ctx1b, ev_div]

                pending.extend(make_evac(hp, ctx2))
                if hp == HP - 1:
                    drain(len(pending))

        att.close()   # release the score PSUM banks for the tail pools
        if dbgt is not None:
            nc.sync.dma_start(out=dbgt["d_ctxT"], in_=ctxT)

        # ---- output projection + LayerNorm ----------------------------
        with (
            tc.tile_pool(name="psO", bufs=1, space="PSUM") as psO,
            tc.tile_pool(name="opool", bufs=3) as op,
            tc.tile_pool(name="lnpool", bufs=4) as lp,
        ):
            # c=0..2 for all q tiles first: runs while the last head-pair's
            # denominator chain (DVE/DMA) is still completing; c=3 + LN after
            ostage = op.tile([128, QT, D], F32, tag="ostage", bufs=1)
            pos = []
            for qt in range(QT):
                po = psO.tile([128, D], F32, tag=f"po{qt}", name="po")
                pos.append(po)
                for c in range(DC - 1):
                    nc.tensor.matmul(
                        po, lhsT=ctxT[:, c, qt * 128:(qt + 1) * 128],
                        rhs=WT["WoT"][:, c, :], start=(c == 0), stop=False)
            for qt in range(QT):
                po = pos[qt]
                c = DC - 1
                nc.tensor.matmul(
                    po, lhsT=ctxT[:, c, qt * 128:(qt + 1) * 128],
                    rhs=WT["WoT"][:, c, :], start=False, stop=zero_bo)
                if not zero_bo:
                    nc.tensor.matmul(po, lhsT=onesrow, rhs=bo_pad,
                                     start=False, stop=True)
                st6 = lp.tile([128, 6], F32, tag="st6")
                nc.vector.bn_stats(out=st6, in_=po)
                mv = lp.tile([128, 2], F32, tag="mv")
                nc.vector.bn_aggr(out=mv, in_=st6)
                sd = lp.tile([128, 1], F32, tag="sd")
                nc.scalar.activation(out=sd, in_=mv[:, 1:2], func=AF.Sqrt,
                                     bias=epsc, scale=1.0)
                nc.vector.reciprocal(out=sd, in_=sd)
                negms = lp.tile([128, 1], F32, tag="negms")
                nc.vector.tensor_scalar(
                    out=negms, in0=mv[:, 0:1], scalar1=sd, scalar2=-1.0,
                    op0=OP.mult, op1=OP.mult)
                if ln_identity:
                    # alternate engines so the 8 scale ops pipeline 2-wide
                    if qt % 2 == 0:
                        nc.scalar.activation(out=ostage[:, qt, :], in_=po,
                                             func=AF.Identity, bias=negms,
                                             scale=sd)
                    else:
                        nc.vector.tensor_scalar(
                            out=ostage[:, qt, :], in0=po, scalar1=sd,
                            scalar2=negms, op0=OP.mult, op1=OP.add)
                else:
                    t1 = op.tile([128, D], F32, tag="t1")
                    nc.scalar.activation(out=t1, in_=po, func=AF.Identity,
                                         bias=negms, scale=sd)
                    t2 = op.tile([128, D], F32, tag="t2")
                    nc.vector.tensor_mul(t2, t1, lng_b)
                    nc.vector.tensor_add(ostage[:, qt, :], t2, lnb_b)
                eng = (nc.sync, nc.scalar)[qt % 2]
                eng.dma_start(out=out[qt * 128:(qt + 1) * 128, :],
                              in_=ostage[:, qt, :])


def make_in_maps(inputs):
    Q = np.asarray(inputs["Q"], np.float32)
    K = np.asarray(inputs["K"], np.float32)
    V = np.asarray(inputs["V"], np.float32)
    mask = np.asarray(inputs["mask"], np.int32)
    temp = float(np.asarray(inputs["temperature"]).reshape(-1)[0])
    tw = np.asarray(inputs["time_weights"], np.float32).reshape(D)
    Wo = np.asarray(inputs["Wo"], np.float32)
    bv = np.asarray(inputs["bv"], np.float32)

    rep = {
        "WqT": np.ascontiguousarray(
            np.asarray(inputs["Wq"], np.float32).T).astype(BF),
        "WkT": np.ascontiguousarray(
            np.asarray(inputs["Wk"], np.float32).T).astype(BF),
        "WvT": np.ascontiguousarray(
            np.asarray(inputs["Wv"], np.float32).T).astype(BF),
        "WoT": np.ascontiguousarray(Wo.T).astype(BF),
        "qaddv": (np.asarray(inputs["bq"], np.float32) + tw),
        "bkiv": (np.asarray(inputs["bk"], np.float32) / temp),
        "invtv": np.full(128, 1.0 / temp, np.float32),
        "gw2": np.tile(np.asarray(inputs["gate_w"],
                                  np.float32).reshape(DK), 2),
        "gb": np.asarray(inputs["gate_b"], np.float32).reshape(1),
        "boeff": (np.asarray(inputs["bo"], np.float32) + Wo @ bv),
        "lngv": np.asarray(inputs["ln_g"], np.float32),
        "lnbv": np.asarray(inputs["ln_b"], np.float32),
    }
    # per-batch transposed bf16 K/V (shared by the two query-half cores)
    KT_b = [np.ascontiguousarray(K[b].T).astype(BF) for b in range(B)]
    VT_b = [np.ascontiguousarray(V[b].T).astype(BF) for b in range(B)]
    mask_f = mask.astype(np.float32)

    in_maps = []
    for c in range(N_CORES):
        b, qh = divmod(c, 2)
        q0 = qh * S_q
        in_maps.append(dict(
            rep,
            QTs=np.ascontiguousarray(Q[b, q0:q0 + S_q, :].T).astype(BF),
            KTs=KT_b[b],
            VTs=VT_b[b],
            MTs=np.ascontiguousarray(
                mask_f[b, 0, q0:q0 + S_q, :].T).astype(BF),
        ))
    return in_maps


def kernel(**inputs):
    ln_identity = (np.all(np.asarray(inputs["ln_g"]) == 1.0)
                   and np.all(np.asarray(inputs["ln_b"]) == 0.0))
    zero_bo = (np.all(np.asarray(inputs["bo"]) == 0.0)
               and np.all(np.asarray(inputs["bv"]) == 0.0))
    nc = build_kernel(ln_identity=ln_identity, zero_bo=zero_bo)
    in_maps = make_in_maps(inputs)
    res = run_bass_kernel_spmd(nc, in_maps, core_ids=list(range(N_CORES)))
    full = np.empty((B, S, D), np.float32)
    for c in range(N_CORES):
        b, qh = divmod(c, 2)
        full[b, qh * S_q:(qh + 1) * S_q, :] = res.results[c]["out"]
    return full


# revision 58
# speedup vs baseline: 1.1837x; 1.1837x over previous
"""Self-contained Trainium2 Bass kernel for EnhancedAutoformerAttention.

Sharding: core c handles batch b=c//2, query half qh=c%2 (1024 queries, all
8 heads). No cross-core reduction; host gather is a pure reshape.

v2 design (ScalarE-exp-bound):
  - Host pre-transposes and pre-casts everything the PE would otherwise
    transpose on chip: W*^T, Q^T/K^T/V^T and the mask arrive as bf16 in the
    exact SBUF layouts the matmuls consume.  No on-chip transposes.
  - ScalarE does ONLY sigmoid (prologue) + the attention exp stream; all
    projection epilogues run on VectorE (tensor_scalar), so the exp stream
    is never interrupted and the ACT table is loaded once.
  - Attention is software-pipelined per (head-pair, kc):
      issue order QK(h0) | PV(prev,h0) | QK(h1) | PV(prev,h1) | exp x2 |
      mask-mult x2 keeps exp back-to-back with 2 score PSUM buffers.
  - Softmax denominators: ones-column in the PV lhsT; reciprocal via the
    DVE approx op (no ScalarE Ln/Exp, no table switches); ctx divide and
    evacuation work is deferred into the next head-pair's loop slack.
"""

from contextlib import ExitStack

import numpy as np
import ml_dtypes

import concourse.bass as bass
import concourse.mybir as mybir
import concourse.tile as tile
from concourse import bacc
from concourse.bass_utils import run_bass_kernel_spmd

dt = mybir.dt
F32, BF16, I32 = dt.float32, dt.bfloat16, dt.int32
AF = mybir.ActivationFunctionType
OP = mybir.AluOpType
BF = ml_dtypes.bfloat16

B, S, D, H, DK = 4, 2048, 512, 8, 64
LN_EPS = 1e-5
N_CORES = 8
S_q, S_kv = 1024, 2048
DC = D // 128          # 4 feature-dim chunks
KC = S_kv // 128       # 16 k-position tiles
ST = S_kv // 128       # 16 v tiles
QT = S_q // 128        # 8 out q tiles
HP = H // 2            # 4 head pairs


def bcast_ap(src: bass.AP, p: int) -> bass.AP:
    """AP reading src (partition dim 1) broadcast to p partitions."""
    return bass.AP(tensor=src.tensor, offset=src.offset,
                   ap=[[0, p]] + list(src.ap[1:]))


def build_kernel(n_devices: int = N_CORES,
                 ln_identity: bool = False, zero_bo: bool = False,
                 dbg: bool = False):
    nc = bacc.Bacc("TRN2", target_bir_lowering=False, debug=False,
                   num_devices=n_devices)
    ein = dict(kind="ExternalInput")
    QTs = nc.dram_tensor("QTs", [D, S_q], BF16, **ein).ap()
    KTs = nc.dram_tensor("KTs", [D, S_kv], BF16, **ein).ap()
    VTs = nc.dram_tensor("VTs", [D, S_kv], BF16, **ein).ap()
    MTs = nc.dram_tensor("MTs", [S_kv, S_q], BF16, **ein).ap()
    Wd = {w: nc.dram_tensor(w, [D, D], BF16, **ein).ap()
          for w in ("WqT", "WkT", "WvT", "WoT")}
    # packed per-partition constants: cols 0:4 bq+tw, 4:8 bk/temp, 8 1/temp,
    # 9 gate_b, 10 gate_w x2 — one DMA instead of six (each pays ~2us receipt)
    constv = nc.dram_tensor("constv", [128, 12], F32, **ein).ap()
    boeff = nc.dram_tensor("boeff", [D], F32, **ein).ap()    # bo + Wo@bv
    lngv = nc.dram_tensor("lngv", [D], F32, **ein).ap()
    lnbv = nc.dram_tensor("lnbv", [D], F32, **ein).ap()
    out = nc.dram_tensor("out", [S_q, D], F32, kind="ExternalOutput").ap()
    dbgt = None
    if dbg:
        eo = dict(kind="ExternalOutput")
        dbgt = {
            "d_qTg": nc.dram_tensor("d_qTg", [128, DC, S_q], BF16, **eo).ap(),
            "d_kT": nc.dram_tensor("d_kT", [128, DC, S_kv], BF16, **eo).ap(),
            "d_vsb": nc.dram_tensor("d_vsb", [128, ST, H, 65], BF16,
                                    **eo).ap(),
            "d_p": nc.dram_tensor("d_p", [128, 2, S_q], BF16, **eo).ap(),
            "d_pm": nc.dram_tensor("d_pm", [128, 2, S_q], BF16, **eo).ap(),
            "d_cf": nc.dram_tensor("d_cf", [2, 65, S_q], F32, **eo).ap(),
            "d_rb": nc.dram_tensor("d_rb", [128, S_q], F32, **eo).ap(),
            "d_ctxT": nc.dram_tensor("d_ctxT", [128, DC, S_q], BF16,
                                     **eo).ap(),
        }

    with tile.TileContext(nc) as tc:
        _body(nc, tc, QTs, KTs, VTs, MTs, Wd, constv,
              boeff, lngv, lnbv, out, ln_identity, zero_bo, dbgt)
    nc.compile()
    return nc


def _body(nc, tc, QTs, KTs, VTs, MTs, Wd, constv,
          boeff, lngv, lnbv, out, ln_identity, zero_bo, dbgt=None):
    with (
        tc.tile_pool(name="persist", bufs=1) as per,
        tc.tile_pool(name="cols", bufs=1) as cols,
    ):
        # ---- small constants: one packed DMA on the gpsimd ring --------
        cz = cols.tile([128, 12], F32, tag="cz")
        nc.gpsimd.dma_start(out=cz, in_=constv)
        qadd = cz[:, 0:4]
        bkic = cz[:, 4:8]
        invt = cz[:, 8:9]
        gbc = cz[:, 9:10]
        gwcol = cz[:, 10:11]
        epsc = cols.tile([128, 1], F32, tag="epsc")
        nc.vector.memset(epsc, LN_EPS)

        # block-diag gate weights (broadcasts gate logit to 64 rows/head)
        ones_bd = cols.tile([128, 128], BF16, tag="ones_bd")
        nc.vector.memset(ones_bd, 0.0)
        nc.vector.memset(ones_bd[0:64, 0:64], 1.0)
        nc.vector.memset(ones_bd[64:128, 64:128], 1.0)
        gwrep = cols.tile([128, 128], BF16, tag="gwrep")
        nc.vector.tensor_scalar_mul(gwrep, ones_bd, gwcol)

        if not ln_identity:
            lng_b = per.tile([128, D], F32, tag="lngb")
            nc.gpsimd.dma_start(out=lng_b, in_=bcast_ap(lngv[None, :], 128))
            lnb_b = per.tile([128, D], F32, tag="lnbb")
            nc.gpsimd.dma_start(out=lnb_b, in_=bcast_ap(lnbv[None, :], 128))
        if not zero_bo:
            onesrow = cols.tile([128, 128], BF16, tag="onesrow")
            nc.vector.memset(onesrow, 0.0)
            nc.vector.memset(onesrow[0:1, :], 1.0)
            bo_pad = cols.tile([128, D], BF16, tag="bo_pad")
            nc.vector.memset(bo_pad, 0.0)
            borow = cols.tile([1, D], F32, tag="borow")
            nc.gpsimd.dma_start(out=borow, in_=boeff[None, :])
            nc.vector.tensor_copy(out=bo_pad[0:1, :], in_=borow)

        # ---- persistent big tensors ------------------------------------
        WT = {w: per.tile([128, DC, D], BF16, tag=w, name=w) for w in Wd}
        maskT = per.tile([128, KC, S_q], BF16, tag="maskT")
        kT = per.tile([128, DC, S_kv], BF16, tag="kT")
        v_sb = per.tile([128, ST, H, 65], BF16, tag="v_sb")
        qT = per.tile([128, DC, S_q], BF16, tag="qT")
        qTg = per.tile([128, DC, S_q], BF16, tag="qTg")
        ctxTr = per.tile([128, DC, S_q], BF16, tag="ctxTr")
        ctxT = per.tile([128, DC, S_q], BF16, tag="ctxT")

        nc.gpsimd.memset(v_sb[:, :, :, 64:65], 1.0)

        # ---- DMAs: spread across the 3 rings in need-order (receipts
        # serialize per ring at ~2us each) ------------------------------
        nc.sync.dma_start(out=WT["WqT"],
                          in_=Wd["WqT"].rearrange("(c p) d -> p c d", p=128))
        att = ExitStack()   # holds psS across the stage-pool close
        with tc.tile_pool(name="stage", bufs=1) as stg:
            QT_s = stg.tile([128, DC, S_q], BF16, tag="QT_s")
            KT_s = stg.tile([128, DC, S_kv], BF16, tag="KT_s")
            VT_s = stg.tile([128, DC, S_kv], BF16, tag="VT_s")
            for sc in range(2):
                sl = slice(sc * 512, (sc + 1) * 512)
                nc.scalar.dma_start(
                    out=QT_s[:, :, sl],
                    in_=QTs[:, sl].rearrange("(c p) q -> p c q", p=128))
            nc.gpsimd.dma_start(
                out=WT["WkT"],
                in_=Wd["WkT"].rearrange("(c p) d -> p c d", p=128))
            for sc in range(4):
                sl = slice(sc * 512, (sc + 1) * 512)
                eng = nc.sync if sc % 2 == 0 else nc.scalar
                eng.dma_start(
                    out=KT_s[:, :, sl],
                    in_=KTs[:, sl].rearrange("(c p) s -> p c s", p=128))
            nc.gpsimd.dma_start(
                out=WT["WvT"],
                in_=Wd["WvT"].rearrange("(c p) d -> p c d", p=128))
            for sc in range(4):
                sl = slice(sc * 512, (sc + 1) * 512)
                eng = nc.sync if sc % 2 == 0 else nc.scalar
                eng.dma_start(
                    out=VT_s[:, :, sl],
                    in_=VTs[:, sl].rearrange("(c p) s -> p c s", p=128))
            for g in range(4):
                sl = slice(g * 4, (g + 1) * 4)
                eng = nc.sync if g % 2 == 0 else nc.scalar
                eng.dma_start(
                    out=maskT[:, sl, :],
                    in_=MTs[g * 512:(g + 1) * 512, :].rearrange(
                        "(kc p) q -> p kc q", p=128))
            nc.gpsimd.dma_start(
                out=WT["WoT"],
                in_=Wd["WoT"].rearrange("(c p) d -> p c d", p=128))

            # ---- projections (prologue; epilogues on VectorE) ----------
            with (
                tc.tile_pool(name="psP", bufs=4, space="PSUM") as psP,
                tc.tile_pool(name="gpool", bufs=2) as gp,
            ):
                # Q projection -> qT (+bq+time_w)
                for sc in range(2):
                    sl = slice(sc * 512, (sc + 1) * 512)
                    for c in range(DC):
                        pq = psP.tile([128, 512], F32, tag="pp", name="pq")
                        for Dc in range(DC):
                            nc.tensor.matmul(
                                pq, lhsT=WT["WqT"][:, Dc, c * 128:(c + 1) * 128],
                                rhs=QT_s[:, Dc, sl],
                                start=(Dc == 0), stop=(Dc == DC - 1))
                        nc.vector.tensor_scalar_add(
                            out=qT[:, c, sl], in0=pq, scalar1=qadd[:, c:c + 1])
                # gate: sigmoid(gw.q + gb) broadcast over dk rows, fold into q
                for c in range(DC):
                    pg = psP.tile([128, S_q], F32, tag="pg", name="pg", bufs=1)
                    for j in range(2):
                        js = slice(j * 512, (j + 1) * 512)
                        nc.tensor.matmul(pg[:, js], lhsT=gwrep,
                                         rhs=qT[:, c, js], start=True, stop=True)
                    gbf = gp.tile([128, S_q], BF16, tag="gbf", name="gbf")
                    nc.scalar.activation(out=gbf, in_=pg, func=AF.Sigmoid,
                                         bias=gbc, scale=1.0)
                    nc.vector.tensor_mul(qTg[:, c, :], qT[:, c, :], gbf)
                # dummy exp: hoists the EXP ACT-table load into the prologue
                scr1 = cols.tile([128, 1], F32, tag="scr1")
                nc.scalar.activation(out=scr1, in_=epsc, func=AF.Exp)
                # K projection -> kT ((k+bk)/temp)
                for sc in range(4):
                    sl = slice(sc * 512, (sc + 1) * 512)
                    for c in range(DC):
                        pk = psP.tile([128, 512], F32, tag="pp", name="pk")
                        for Dc in range(DC):
                            nc.tensor.matmul(
                                pk, lhsT=WT["WkT"][:, Dc, c * 128:(c + 1) * 128],
                                rhs=KT_s[:, Dc, sl],
                                start=(Dc == 0), stop=(Dc == DC - 1))
                        nc.vector.tensor_scalar(
                            out=kT[:, c, sl], in0=pk,
                            scalar1=invt, scalar2=bkic[:, c:c + 1],
                            op0=OP.mult, op1=OP.add)
                # V projection -> v_sb natural [s, h, dk] (bv folded into bo)
                for st in range(ST - 4):
                    pv = psP.tile([128, 512], F32, tag="pp", name="pv")
                    for Dc in range(DC):
                        nc.tensor.matmul(
                            pv, lhsT=VT_s[:, Dc, st * 128:(st + 1) * 128],
                            rhs=WT["WvT"][:, Dc, :],
                            start=(Dc == 0), stop=(Dc == DC - 1))
                    nc.vector.tensor_copy(
                        out=v_sb[:, st, :, 0:64],
                        in_=pv.rearrange("p (h d) -> p h d", h=H))

            # prologue PSUM is now free: open the score pool early and run
            # the last 4 V chunks in its banks — a ~3.4us dense matmul burst
            # (the PE warmup) that is real work instead of dummies
            psS = att.enter_context(
                tc.tile_pool(name="psS", bufs=1, space="PSUM"))
            vsc = [psS.tile([128, S_q], F32, tag=f"sc{i}", name=f"vsc{i}")
                   for i in range(2)]
            for i, st in enumerate(range(ST - 4, ST)):
                sl = vsc[i % 2][:, (i // 2) * 512:(i // 2 + 1) * 512]
                for Dc in range(DC):
                    nc.tensor.matmul(
                        sl, lhsT=VT_s[:, Dc, st * 128:(st + 1) * 128],
                        rhs=WT["WvT"][:, Dc, :],
                        start=(Dc == 0), stop=(Dc == DC - 1))
                nc.vector.tensor_copy(
                    out=v_sb[:, st, :, 0:64],
                    in_=sl.rearrange("p (h d) -> p h d", h=H))

        if dbgt is not None:
            nc.sync.dma_start(out=dbgt["d_qTg"], in_=qTg)
            nc.sync.dma_start(out=dbgt["d_kT"], in_=kT)
            nc.sync.dma_start(out=dbgt["d_vsb"], in_=v_sb)

        # ---- attention core (exp-paced pipeline) -----------------------
        with (
            tc.tile_pool(name="psC", bufs=1, space="PSUM") as psC,
            tc.tile_pool(name="pp", bufs=4) as pp,
            tc.tile_pool(name="rp", bufs=2) as rp,
            tc.tile_pool(name="rd", bufs=2, space="DRAM") as rd,
        ):
            pending = []   # deferred DVE/DMA work, drained 1 task per kc

            def drain(n):
                for _ in range(min(n, len(pending))):
                    pending.pop(0)()

            for hp in range(HP):
                ctx2 = [psC.tile([65, S_q], F32, tag=f"ctx{i}", name=f"ctx{i}")
                        for i in range(2)]
                pm_prev = None
                for kc in range(KC):
                    sc2 = [psS.tile([128, S_q], F32, tag=f"sc{i}",
                                    name=f"sc{i}") for i in range(2)]
                    kcs = slice(kc * 128, (kc + 1) * 128)
                    # PE: QK(h0) | PV(prev,h0) | QK(h1) | PV(prev,h1)
                    for half in range(2):
                        hs = slice(half * 64, (half + 1) * 64)
                        for j in range(2):
                            js = slice(j * 512, (j + 1) * 512)
                            nc.tensor.matmul(
                                sc2[half][:, js], lhsT=kT[hs, hp, kcs],
                                rhs=qTg[hs, hp, js], start=True, stop=True)
                        if pm_prev is not None:
                            for j in range(2):
                                js = slice(j * 512, (j + 1) * 512)
                                nc.tensor.matmul(
                                    ctx2[half][:, js],
                                    lhsT=v_sb[:, kc - 1, 2 * hp + half, :],
                                    rhs=pm_prev[half][:, js],
                                    start=(kc - 1 == 0), stop=(kc - 1 == KC - 1))
                    # ScalarE: exp
                    p2 = [pp.tile([128, S_q], BF16, tag=f"p{i}", name=f"p{i}",
                                  bufs=6) for i in range(2)]
                    pm2 = [pp.tile([128, S_q], BF16, tag=f"pm{i}",
                                   name=f"pm{i}", bufs=6) for i in range(2)]
                    for half in range(2):
                        nc.scalar.activation(out=p2[half], in_=sc2[half],
                                             func=AF.Exp)
                        nc.vector.tensor_mul(pm2[half], p2[half],
                                             maskT[:, kc, :])
                    if dbgt is not None and hp == 0 and kc == 0:
                        for half in range(2):
                            nc.sync.dma_start(out=dbgt["d_p"][:, half, :],
                                              in_=p2[half])
                            nc.sync.dma_start(out=dbgt["d_pm"][:, half, :],
                                              in_=pm2[half])
                    pm_prev = pm2
                    drain(1)
                # final PV for kc=KC-1
                for half in range(2):
                    for j in range(2):
                        js = slice(j * 512, (j + 1) * 512)
                        nc.tensor.matmul(
                            ctx2[half][:, js],
                            lhsT=v_sb[:, KC - 1, 2 * hp + half, :],
                            rhs=pm_prev[half][:, js],
                            start=False, stop=True)

                # evacuate ctx+den, reciprocal, divide — deferred into the
                # next head-pair's loop slack (DVE/DMA only, no ScalarE)
                def make_evac(hp, ctx2):
                    # ctx PSUM must be read out NOW: the next head-pair's
                    # first PV (start=True) reuses these banks, and a
                    # deferred reader issued after that writer would race.
                    cf = []
                    rdt = rd.tile([2, S_q], F32, tag="rdt", name="rdt")
                    for half in range(2):
                        c = rp.tile([65, S_q], F32, tag=f"cf{half}", name="cf")
                        if hp == HP - 1 and half == 0:
                            # ScalarE is free after the last exp — parallel
                            # evacuation shortens the tail chain
                            nc.scalar.copy(out=c, in_=ctx2[half][0:65, :])
                        else:
                            nc.vector.tensor_copy(out=c,
                                                  in_=ctx2[half][0:65, :])
                        cf.append(c)
                        # den row to DRAM as soon as this half is evacuated
                        nc.sync.dma_start(out=rdt[half:half + 1, :],
                                          in_=c[64:65, :])
                    if dbgt is not None and hp == 0:
                        for half in range(2):
                            nc.sync.dma_start(out=dbgt["d_cf"][half],
                                              in_=cf[half])
                    ctmp = [None]

                    def ev_ctx0():
                        nc.vector.tensor_copy(out=ctxTr[0:64, hp, :],
                                              in_=cf[0][0:64, :])

                    def ev_ctx1():
                        ctmp[0] = rp.tile([64, S_q], BF16, tag="ctmp",
                                          name="ctmp")
                        nc.vector.tensor_copy(out=ctmp[0], in_=cf[1][0:64, :])

                    def ev_ctx1b():
                        nc.sync.dma_start(out=ctxTr[64:128, hp, :], in_=ctmp[0])

                    def ev_rb():
                        rb = rp.tile([128, S_q], F32, tag="rb", name="rb",
                                     bufs=1)
                        nc.sync.dma_start(out=rb[0:64, :],
                                          in_=bcast_ap(rdt[0:1, :], 64))
                        nc.sync.dma_start(out=rb[64:128, :],
                                          in_=bcast_ap(rdt[1:2, :], 64))
                        cf.append(rb)

                    def ev_div():
                        rb = cf[2]
                        rbr = rp.tile([128, S_q], F32, tag="rbr", name="rbr",
                                      bufs=1)
                        nc.vector.reciprocal_approx_fast(out=rbr, in_=rb)
                        nc.vector.tensor_mul(ctxT[:, hp, :], ctxTr[:, hp, :],
                                             rbr)
                        if dbgt is not None and hp == 0:
                            nc.sync.dma_start(out=dbgt["d_rb"], in_=rbr)
                    return [ev_rb, ev_ctx0, ev_ctx1, ev_ctx1b, ev_div]

                pending.extend(make_evac(hp, ctx2))
                if hp == HP - 1:
                    drain(len(pending))

        att.close()   # release the score PSUM banks for the tail pools
        if dbgt is not None:
            nc.sync.dma_start(out=dbgt["d_ctxT"], in_=ctxT)

        # ---- output projection + LayerNorm ----------------------------
        with (
            tc.tile_pool(name="psO", bufs=1, space="PSUM") as psO,
            tc.tile_pool(name="opool", bufs=3) as op,
            tc.tile_pool(name="lnpool", bufs=4) as lp,
        ):
            # c=0..2 for all q tiles first: runs while the last head-pair's
            # denominator chain (DVE/DMA) is still completing; c=3 + LN after
            ostage = op.tile([128, QT, D], F32, tag="ostage", bufs=1)
            pos = []
            for qt in range(QT):
                po = psO.tile([128, D], F32, tag=f"po{qt}", name="po")
                pos.append(po)
                for c in range(DC - 1):
                    nc.tensor.matmul(
                        po, lhsT=ctxT[:, c, qt * 128:(qt + 1) * 128],
                        rhs=WT["WoT"][:, c, :], start=(c == 0), stop=False)
            for qt in range(QT):
                po = pos[qt]
                c = DC - 1
                nc.tensor.matmul(
                    po, lhsT=ctxT[:, c, qt * 128:(qt + 1) * 128],
                    rhs=WT["WoT"][:, c, :], start=False, stop=zero_bo)
                if not zero_bo:
                    nc.tensor.matmul(po, lhsT=onesrow, rhs=bo_pad,
                                     start=False, stop=True)
                st6 = lp.tile([128, 6], F32, tag="st6")
                nc.vector.bn_stats(out=st6, in_=po)
                mv = lp.tile([128, 2], F32, tag="mv")
                nc.vector.bn_aggr(out=mv, in_=st6)
                sd = lp.tile([128, 1], F32, tag="sd")
                nc.scalar.activation(out=sd, in_=mv[:, 1:2], func=AF.Sqrt,
                                     bias=epsc, scale=1.0)
                nc.vector.reciprocal(out=sd, in_=sd)
                negms = lp.tile([128, 1], F32, tag="negms")
                nc.vector.tensor_scalar(
                    out=negms, in0=mv[:, 0:1], scalar1=sd, scalar2=-1.0,
                    op0=OP.mult, op1=OP.mult)
                if ln_identity:
                    # alternate engines so the 8 scale ops pipeline 2-wide
                    if qt % 2 == 0:
                        nc.scalar.activation(out=ostage[:, qt, :], in_=po,
                                             func=AF.Identity, bias=negms,
                                             scale=sd)
                    else:
                        nc.vector.tensor_scalar(
                            out=ostage[:, qt, :], in0=po, scalar1=sd,
                            scalar2=negms, op0=OP.mult, op1=OP.add)
                else:
                    t1 = op.tile([128, D], F32, tag="t1")
                    nc.scalar.activation(out=t1, in_=po, func=AF.Identity,
                                         bias=negms, scale=sd)
                    t2 = op.tile([128, D], F32, tag="t2")
                    nc.vector.tensor_mul(t2, t1, lng_b)
                    nc.vector.tensor_add(ostage[:, qt, :], t2, lnb_b)
                eng = (nc.sync, nc.scalar)[qt % 2]
                eng.dma_start(out=out[qt * 128:(qt + 1) * 128, :],
                              in_=ostage[:, qt, :])


def _pack_consts(inputs, tw, temp):
    """[128, 12] f32: cols 0:4 bq+tw, 4:8 bk/temp, 8 1/temp, 9 gate_b,
    10 gate_w tiled x2 (column layouts match the on-chip slices)."""
    cz = np.zeros((128, 12), np.float32)
    qaddv = np.asarray(inputs["bq"], np.float32) + tw
    bkiv = np.asarray(inputs["bk"], np.float32) / temp
    cz[:, 0:4] = qaddv.reshape(DC, 128).T
    cz[:, 4:8] = bkiv.reshape(DC, 128).T
    cz[:, 8] = 1.0 / temp
    cz[:, 9] = float(np.asarray(inputs["gate_b"]).reshape(-1)[0])
    cz[:, 10] = np.tile(np.asarray(inputs["gate_w"],
                                   np.float32).reshape(DK), 2)
    return cz


def make_in_maps(inputs):
    Q = np.asarray(inputs["Q"], np.float32)
    K = np.asarray(inputs["K"], np.float32)
    V = np.asarray(inputs["V"], np.float32)
    mask = np.asarray(inputs["mask"], np.int32)
    temp = float(np.asarray(inputs["temperature"]).reshape(-1)[0])
    tw = np.asarray(inputs["time_weights"], np.float32).reshape(D)
    Wo = np.asarray(inputs["Wo"], np.float32)
    bv = np.asarray(inputs["bv"], np.float32)

    rep = {
        "WqT": np.ascontiguousarray(
            np.asarray(inputs["Wq"], np.float32).T).astype(BF),
        "WkT": np.ascontiguousarray(
            np.asarray(inputs["Wk"], np.float32).T).astype(BF),
        "WvT": np.ascontiguousarray(
            np.asarray(inputs["Wv"], np.float32).T).astype(BF),
        "WoT": np.ascontiguousarray(Wo.T).astype(BF),
        "constv": _pack_consts(inputs, tw, temp),
        "boeff": (np.asarray(inputs["bo"], np.float32) + Wo @ bv),
        "lngv": np.asarray(inputs["ln_g"], np.float32),
        "lnbv": np.asarray(inputs["ln_b"], np.float32),
    }
    # per-batch transposed bf16 K/V (shared by the two query-half cores)
    KT_b = [np.ascontiguousarray(K[b].T).astype(BF) for b in range(B)]
    VT_b = [np.ascontiguousarray(V[b].T).astype(BF) for b in range(B)]
    mask_f = mask.astype(np.float32)

    in_maps = []
    for c in range(N_CORES):
        b, qh = divmod(c, 2)
        q0 = qh * S_q
        in_maps.append(dict(
            rep,
            QTs=np.ascontiguousarray(Q[b, q0:q0 + S_q, :].T).astype(BF),
            KTs=KT_b[b],
            VTs=VT_b[b],
            MTs=np.ascontiguousarray(
                mask_f[b, 0, q0:q0 + S_q, :].T).astype(BF),
        ))
    return in_maps


def kernel(**inputs):
    ln_identity = (np.all(np.asarray(inputs["ln_g"]) == 1.0)
                   and np.all(np.asarray(inputs["ln_b"]) == 0.0))
    zero_bo = (np.all(np.asarray(inputs["bo"]) == 0.0)
               and np.all(np.asarray(inputs["bv"]) == 0.0))
    nc = build_kernel(ln_identity=ln_identity, zero_bo=zero_bo)
    in_maps = make_in_maps(inputs)
    res = run_bass_kernel_spmd(nc, in_maps, core_ids=list(range(N_CORES)))
    full = np.empty((B, S, D), np.float32)
    for c in range(N_CORES):
        b, qh = divmod(c, 2)
        full[b, qh * S_q:(qh + 1) * S_q, :] = res.results[c]["out"]
    return full
